# revision 24
# baseline (speedup 1.0000x reference)
"""CircleLoss (nn_CircleLoss_55482387529741) Trainium2 Bass kernel.

Math (B=8192, D=128, margin m=0.25, gamma=256=16^2):
  a = l2norm(A) rows, b = l2norm(B) rows, s_ij = a_i . b_j
  logit_neg = g*relu(s-m)*(s+m)  ==>  exp(logit_neg) = exp(max(16*s, 4)^2 - 16)
  lse_pos_i = (w-12)(w-4) with w = min(16*s_ii, 12)
  loss_i = softplus(lse_pos_i + log(sum_{j!=i} exp(logit_neg_ij)))
  out = mean(loss)

Distribution: a-rows sharded 8 x 1024 across cores. Each core computes a
[8192 x 1024] "flipped" sim slab (partitions = b-rows, free = its a-rows).
B is rotated per core on host so the diagonal lands in local b-blocks 0..7.

Per 128-row b-block: PE matmul -> psum r; then one of two elementwise routes:
  route D (DVE): one fused custom op emits bf16 BITS of exp(z-16) via the
    Schraudolph trick: i16 = round(sq(max(r*invb, 4))*A16 + (B16-16*A16));
    bitcast i16 -> bf16 is e. One DVE pass, no ACT.
  route A (ACT): Relu(r*invb-4) -> Square(y+4) -> Exp(q-16), exact, 3 passes.
Routes are split ~47/17 to run DVE and ACT at full duty in parallel; the PE
ones-matmul accumulates row sums S over all 64 blocks (216 ns/512-col issue
rate, LDW hidden). b-norm prep: GPSIMD squares (prep window only - its SBUF
port contends with DVE), PE selector-matmul partition reduce, short DRAM
roundtrip. No GPSIMD work once the hot loop runs. Epilogue ln() uses the
inverse bit trick (bias-corrected); S reshape via PE row transposes and the
losses go out partition-major (host reorders) - no 4-byte scatter DMAs.
"""

import sys

for _p in ("/opt/trn_rl_repo",):
    if _p not in sys.path:
        sys.path.append(_p)

import numpy as np

import concourse.bass as bass
from concourse import bacc
import concourse.mybir as mybir
import concourse.tile as tile
from concourse.bass_utils import run_bass_kernel_spmd
from concourse.masks import make_identity

F32 = mybir.dt.float32
BF16 = mybir.dt.bfloat16
I16 = mybir.dt.int16
I32 = mybir.dt.int32
AF = mybir.ActivationFunctionType
OP = mybir.AluOpType

B = 8192
D = 128
NCORES = 8
MPC = B // NCORES  # 1024 a-rows per core
NB = B // 128  # 64 b-blocks
NA = MPC // 128  # 8 a-tiles

A16 = 128.0 / float(np.log(2.0))  # bf16 Schraudolph scale
BOFF = 127.0 * 128.0 - 16.0 * A16  # bits offset incl. the -16 in exp(z-16)
A32 = float(2**23) / float(np.log(2.0))
LNC = 127.0 * 2**23 / A32  # subtract after bits/A32
LN_BIAS = 0.0397  # E[ln(1+m) - m*ln2] over uniform mantissa

# blocks handled by the exact ACT route (rest use the DVE bit-trick route).
# diag blocks 0..7 stay on DVE so the antieye fix never crosses engines.
N_ACT = 19
ACT_BLOCKS = sorted({8 + int(round(i * 55 / max(N_ACT - 1, 1))) for i in range(N_ACT)})

_cache = {}

# test-harness hooks (default off; kernel() stays self-contained)
TRACE = False
TRACE_DIR = None
LAST_RESULTS = None


def _get_custom_op():
    """out_i16 = round(sq(maxx(in0*s0, s1))*imm2 + C3), C3 latched from in1."""
    from concourse import dve_ops
    from concourse.dve_spec import (
        Spec, Src0, C0, C1, C2, C3, maxx, sq, lower, _spill_c3_to_src1,
    )
    from concourse.dve_spec import _has_src1 as has_src1
    from concourse.dve_uop import DveOpSpec

    name = "SCHRAUDOLPH_CIRCLE"
    for o in dve_ops.OPS:
        if o.name == name:
            return o

    def _ref(in0, in1, s0, s1, imm2):
        z = np.square(
            np.maximum(in0.astype(np.float32) * np.float32(s0), np.float32(s1))
        )
        return (z * np.float32(imm2) + in1.astype(np.float32)).astype(np.float32)

    spec = Spec(
        body=_spill_c3_to_src1(sq(maxx(Src0 * C0, C1)) * C2 + C3), reference=_ref
    )
    opcode = dve_ops._CUSTOM_DVE_ROW_BASE + len(dve_ops.OPS)
    assert opcode < 0x20
    shas = {}
    for ver in ("v3", "v4"):
        try:
            shas[ver] = DveOpSpec(
                name=name,
                opcode=opcode,
                uops=lower(spec, ver=ver),
                rd1_en=has_src1(spec),
            ).sha(ver)
        except Exception:
            pass
    op = dve_ops.DveOp(name, spec, subdim=False, uops_sha=shas)
    dve_ops.OPS.append(op)
    dve_ops.CUSTOM_DVE_SPECS[name] = spec
    dve_ops._SUB_OPCODE_FOR_NAME[name] = opcode
    return op


def _build():
    if "nc" in _cache:
        return _cache["nc"]
    op = _get_custom_op()
    nc = bacc.Bacc("TRN2", target_bir_lowering=False)

    a_in = nc.declare_dram_parameter("a_shard", [MPC, D], F32, isOutput=False)
    bT_in = nc.declare_dram_parameter("bT", [D, B], BF16, isOutput=False)
    bd_in = nc.declare_dram_parameter("b_diag", [MPC, D], F32, isOutput=False)
    out = nc.declare_dram_parameter("losses", [MPC], F32, isOutput=True)
    ssb_scr = nc.dram_tensor("ssb_scratch", [B], F32)
    out_pm = out.rearrange("(p m) -> p m", p=128)  # contiguous per partition

    with tile.TileContext(nc) as tc:
        with (
            tc.tile_pool(name="consts", bufs=1) as consts,
            tc.tile_pool(name="big", bufs=1) as big,
            tc.tile_pool(name="aprep", bufs=1) as aprep,
            tc.tile_pool(name="bsq", bufs=2) as bsqp,
            tc.tile_pool(name="epool", bufs=5) as epool,
            tc.tile_pool(name="ypool", bufs=2) as ypool,
            tc.tile_pool(name="stats", bufs=1) as stats,
            tc.tile_pool(name="psim", bufs=3, space="PSUM") as psim,
            tc.tile_pool(name="psacc", bufs=1, space="PSUM") as psacc,
        ):
            # ---- DMAs: a_big and bT0 lead (they gate both prep chains);
            # bT1-3/b_diag queue via the otherwise-idle gpsimd stream and
            # bT4-7 via the scalar stream mid-a-prep so the early transfers
            # get the HBM bandwidth.
            a_in_pm = a_in.rearrange("(i p) d -> p i d", p=128)
            a_big = aprep.tile([128, NA, D], F32, tag="a_stage")
            nc.sync.dma_start(out=a_big[:, 0:4, :], in_=a_in_pm[:, 0:4, :])
            nc.sync.dma_start(out=a_big[:, 4:8, :], in_=a_in_pm[:, 4:8, :])
            bT = big.tile([128, B], BF16, tag="bT")
            nc.sync.dma_start(out=bT[:, 0:512], in_=bT_in[:, 0:512])
            nc.sync.dma_start(out=bT[:, 512:1024], in_=bT_in[:, 512:1024])
            b_diag = aprep.tile([128, NA, D], F32, tag="b_diag")
            for g in range(1, 4):
                nc.gpsimd.dma_start(
                    out=bT[:, g * 1024:(g + 1) * 1024],
                    in_=bT_in[:, g * 1024:(g + 1) * 1024],
                )
            nc.gpsimd.dma_start(
                out=b_diag, in_=bd_in.rearrange("(i p) d -> p i d", p=128)
            )

            # ---- constants ----
            eye = consts.tile([128, 128], F32, tag="eye")
            make_identity(nc, eye)
            antieye = consts.tile([128, 128], I16, tag="antieye")
            nc.vector.tensor_scalar(
                out=antieye, in0=eye, scalar1=-1.0, scalar2=1.0,
                op0=OP.mult, op1=OP.add,
            )
            ones = consts.tile([128, 1], BF16, tag="ones")
            nc.vector.memset(ones, 1.0)
            one1 = consts.tile([1, 1], F32, tag="one1")
            nc.vector.memset(one1, 1.0)
            bcol = consts.tile([128, 1], F32, tag="bcol")
            nc.vector.memset(bcol, BOFF)
            b_m4 = consts.tile([128, 1], F32, tag="b_m4")
            nc.vector.memset(b_m4, -4.0)
            b_p4 = consts.tile([128, 1], F32, tag="b_p4")
            nc.vector.memset(b_p4, 4.0)
            b_m16 = consts.tile([128, 1], F32, tag="b_m16")
            nc.vector.memset(b_m16, -16.0)
            wsel = consts.tile([128, 1024], BF16, tag="wsel")
            nc.vector.memset(wsel, 0.0)
            for g in range(4):
                nc.vector.memset(wsel[:, g * 128 + g:g * 128 + g + 1], 1.0)

            # ---- persistent tensors ----
            aT = big.tile([128, MPC], BF16, tag="aT")
            invb = stats.tile([128, NB], F32, tag="invb")
            rd = stats.tile([128, NA], F32, tag="rd")

            # ---- b squares for the first half (DVE, fills its idle time) --
            bsqs = []
            for g in range(8):
                bsq = bsqp.tile([128, 1024], BF16, tag="b_sq")
                bsqs.append(bsq)

            def bsq_mm(g):
                nc.vector.tensor_mul(
                    bsqs[g], bT[:, g * 1024:(g + 1) * 1024],
                    bT[:, g * 1024:(g + 1) * 1024],
                )

            for g in range(2):
                bsq_mm(g)

            # ---- a prep, pipelined per 128-row tile ----
            # Square+rowsum fused on ACT via accum_out; reciprocal and the
            # broadcast scale on DVE; transposes on PE.
            asq = aprep.tile([128, NA, D], F32, tag="a_sq")
            ssa = stats.tile([128, NA], F32, tag="ssa")
            ra = stats.tile([128, NA], F32, tag="ra")
            inva16 = stats.tile([128, NA], F32, tag="inva16")
            a16 = aprep.tile([128, NA, D], F32, tag="a16")
            pt = psim.tile([128, 1024], F32, tag="sim")
            for i in range(NA):
                if i == 4:
                    bsq_mm(2)
                    bsq_mm(3)
                    for g in range(4, 8):
                        nc.scalar.dma_start(
                            out=bT[:, g * 1024:(g + 1) * 1024],
                            in_=bT_in[:, g * 1024:(g + 1) * 1024],
                        )
                nc.scalar.activation(
                    out=asq[:, i, :], in_=a_big[:, i, :], func=AF.Square,
                    accum_out=ssa[:, i:i + 1],
                )
                nc.vector.reciprocal(out=ra[:, i:i + 1], in_=ssa[:, i:i + 1])
                nc.scalar.activation(
                    out=inva16[:, i:i + 1], in_=ra[:, i:i + 1],
                    func=AF.Sqrt, scale=256.0,
                )
                nc.vector.tensor_mul(
                    a16[:, i, :], a_big[:, i, :],
                    inva16[:, i:i + 1].to_broadcast((128, D)),
                )
                nc.tensor.transpose(
                    pt[:, i * 128:(i + 1) * 128], a16[:, i, :], eye
                )
            nc.scalar.copy(out=aT[:, 0:512], in_=pt[:, 0:512])
            nc.scalar.copy(out=aT[:, 512:1024], in_=pt[:, 512:1024])

            # ---- b-norm partition reduce: selector matmuls on PE ----
            ssb_ps = psim.tile([128, 1024], F32, tag="sim")

            def wk_mm(g, start, stop):
                # selector column g%4 -> chunk sums land on psum row g%4
                q = g % 4
                for h in range(2):
                    nc.tensor.matmul(
                        ssb_ps[:, h * 512:(h + 1) * 512],
                        wsel[:, q * 128:(q + 1) * 128],
                        bsqs[g][:, h * 512:(h + 1) * 512],
                        start=start, stop=stop, skip_group_check=True,
                    )

            # S accumulator region; its partitions 1-127 are borrowed for
            # the invb chunk-0 reshape transposes before the first ones-mm.
            Sbig = psacc.tile([128, MPC], F32, tag="S")
            S_ps = Sbig[0:1, :]

            for g in range(4):
                wk_mm(g, start=(g == 0), stop=(g == 3))
            ssbA_sb = stats.tile([4, 1024], F32, tag="ssbA_sb")
            nc.scalar.copy(out=ssbA_sb, in_=ssb_ps[0:4, :])
            # invb for blocks 0-7 via 8 PE row transposes (no DRAM gather)
            for m in range(8):
                nc.tensor.transpose(
                    Sbig[:, m:m + 1], ssbA_sb[0:1, m * 128:(m + 1) * 128], one1
                )
            ssbA_pm0 = stats.tile([128, 8], F32, tag="ssbA_pm0")
            nc.scalar.copy(out=ssbA_pm0, in_=Sbig[:, 0:8])
            rbA0 = stats.tile([128, 8], F32, tag="rbA0")
            nc.vector.reciprocal(out=rbA0, in_=ssbA_pm0)
            nc.scalar.activation(out=invb[:, 0:8], in_=rbA0, func=AF.Sqrt)
            # blocks 8-31 via the DRAM roundtrip (latency hidden)
            nc.sync.dma_start(
                out=ssb_scr[0:4096].rearrange("(q j) -> q j", q=4), in_=ssbA_sb
            )
            ssbA_pm1 = stats.tile([128, 24], F32, tag="ssbA_pm1")
            nc.sync.dma_start(
                out=ssbA_pm1,
                in_=ssb_scr[1024:4096].rearrange("(m p) -> p m", p=128),
            )
            rbA1 = stats.tile([128, 24], F32, tag="rbA1")
            nc.vector.reciprocal(out=rbA1, in_=ssbA_pm1)
            nc.scalar.activation(out=invb[:, 8:32], in_=rbA1, func=AF.Sqrt)

            for g in range(4, 8):
                bsq_mm(g)
            # rd = rowsum(a_big * b_diag) * inva16 (applied in the epilogue):
            # raw diagonal sim in row layout, decoupled from the a16 chain
            rdp = aprep.tile([128, NA, D], F32, tag="rdp")
            nc.vector.tensor_mul(rdp, a_big, b_diag)
            nc.vector.tensor_reduce(
                out=rd, in_=rdp, axis=mybir.AxisListType.X, op=OP.add
            )
            for g in range(4, 8):
                wk_mm(g, start=(g == 4), stop=(g == 7))
            ssbB_sb = stats.tile([4, 1024], F32, tag="ssbB_sb")
            nc.scalar.copy(out=ssbB_sb, in_=ssb_ps[0:4, :])
            nc.sync.dma_start(
                out=ssb_scr[4096:8192].rearrange("(q j) -> q j", q=4),
                in_=ssbB_sb,
            )
            ssbB_pm = stats.tile([128, 32], F32, tag="ssbB_pm")
            nc.sync.dma_start(
                out=ssbB_pm,
                in_=ssb_scr[4096:8192].rearrange("(m p) -> p m", p=128),
            )

            act_set = set(ACT_BLOCKS)
            movs = {}

            def elementwise(m, ps):
                if m in act_set:
                    y = ypool.tile([128, MPC], F32, tag="y")
                    nc.scalar.activation(
                        out=y, in_=ps, func=AF.Relu,
                        scale=invb[:, m:m + 1], bias=b_m4,
                    )
                    q = ypool.tile([128, MPC], F32, tag="q")
                    nc.scalar.activation(out=q, in_=y, func=AF.Square, bias=b_p4)
                    e = epool.tile([128, MPC], BF16, tag="ea")
                    nc.scalar.activation(out=e, in_=q, func=AF.Exp, bias=b_m16)
                    movs[m] = e
                else:
                    eb = epool.tile([128, MPC], I16, tag="eb")
                    nc.vector._custom_dve(
                        op, out=eb, in0=ps, in1=bcol,
                        s0=invb[:, m:m + 1], s1=4.0, imm2=A16,
                    )
                    if m < NA:
                        sl = slice(m * 128, (m + 1) * 128)
                        nc.vector.tensor_mul(eb[:, sl], eb[:, sl], antieye)
                    movs[m] = eb.bitcast(BF16)

            # ---- main loop over 64 b-blocks, ones-matmuls batched by 4 ----
            for grp in range(16):
                ms = range(grp * 4, grp * 4 + 4)
                if grp == 1:
                    # second half of b-norm prep; data is ready long before
                    # the engine streams reach this point, so no stall. The
                    # throwaway Exp preloads the exp table set ahead of the
                    # first ACT-route block.
                    rbB = stats.tile([128, 32], F32, tag="rbB")
                    nc.vector.reciprocal(out=rbB, in_=ssbB_pm)
                    nc.scalar.activation(out=invb[:, 32:64], in_=rbB, func=AF.Sqrt)
                    warm = stats.tile([128, 1], F32, tag="warm")
                    nc.scalar.activation(out=warm, in_=bcol, func=AF.Exp, scale=0.0)
                for m in ms:
                    ps = psim.tile([128, MPC], F32, tag="sim")
                    for h in range(2):
                        nc.tensor.matmul(
                            ps[:, h * 512:(h + 1) * 512],
                            bT[:, m * 128:(m + 1) * 128],
                            aT[:, h * 512:(h + 1) * 512],
                            start=True, stop=True,
                        )
                    elementwise(m, ps)
                for m in ms:
                    mov = movs.pop(m)
                    for h in range(2):
                        nc.tensor.matmul(
                            S_ps[:, h * 512:(h + 1) * 512],
                            ones,
                            mov[:, h * 512:(h + 1) * 512],
                            start=(m == 0), stop=(m == NB - 1),
                            skip_group_check=True,
                        )

            # ---- epilogue: per-row losses ----
            S_sb = stats.tile([1, MPC], F32, tag="S_sb")
            nc.scalar.copy(out=S_sb, in_=S_ps)
            # reshape [1, 1024] -> [128, 8] with 8 PE row transposes (no DMA)
            sT = psim.tile([128, 1024], F32, tag="sim")
            for m in range(NA):
                nc.tensor.transpose(
                    sT[:, m:m + 1], S_sb[0:1, m * 128:(m + 1) * 128], one1
                )
            Srs = stats.tile([128, NA], F32, tag="Srs")
            nc.scalar.copy(out=Srs, in_=sT[:, 0:NA])
            # lse = ln(S) via inverse bit trick (+mean-bias fix folded into +48)
            lse = stats.tile([128, NA], F32, tag="lse")
            nc.vector.tensor_scalar(
                out=lse, in0=Srs.bitcast(I32), scalar1=1.0 / A32, scalar2=-LNC,
                op0=OP.mult, op1=OP.add,
            )
            sdiag0 = stats.tile([128, NA], F32, tag="sdiag0")
            nc.vector.tensor_mul(sdiag0, rd, inva16)
            sdiag = stats.tile([128, NA], F32, tag="sdiag")
            nc.vector.tensor_mul(sdiag, sdiag0, invb[:, 0:NA])
            w = stats.tile([128, NA], F32, tag="w")
            nc.vector.tensor_scalar(
                out=w, in0=sdiag, scalar1=12.0, scalar2=None, op0=OP.min
            )
            lpr = stats.tile([128, NA], F32, tag="lpr")
            nc.vector.scalar_tensor_tensor(
                out=lpr, in0=w, scalar=16.0, in1=w, op0=OP.subtract, op1=OP.mult
            )
            t = stats.tile([128, NA], F32, tag="t")
            nc.vector.scalar_tensor_tensor(
                out=t, in0=lpr, scalar=48.0 + LN_BIAS, in1=lse,
                op0=OP.add, op1=OP.add,
            )
            abst = stats.tile([128, NA], F32, tag="abst")
            nc.scalar.activation(out=abst, in_=t, func=AF.Abs)
            u = stats.tile([128, NA], F32, tag="u")
            nc.scalar.activation(out=u, in_=abst, func=AF.Exp, scale=-1.0)
            up1 = stats.tile([128, NA], F32, tag="up1")
            nc.vector.tensor_scalar(
                out=up1, in0=u, scalar1=1.0, scalar2=None, op0=OP.add
            )
            v = stats.tile([128, NA], F32, tag="v")
            nc.vector.tensor_scalar(
                out=v, in0=up1.bitcast(I32), scalar1=1.0 / A32, scalar2=-LNC,
                op0=OP.mult, op1=OP.add,
            )
            loss = stats.tile([128, NA], F32, tag="loss")
            nc.vector.scalar_tensor_tensor(
                out=loss, in0=t, scalar=0.0, in1=v, op0=OP.max, op1=OP.add
            )
            nc.sync.dma_start(out=out_pm, in_=loss)

    nc.finalize()
    _cache["nc"] = nc
    return nc


def kernel(embeddings_a: np.ndarray, embeddings_b: np.ndarray) -> np.ndarray:
    import ml_dtypes

    nc = _build()
    A = np.ascontiguousarray(embeddings_a, dtype=np.float32)
    Bm = np.ascontiguousarray(embeddings_b, dtype=np.float32)
    BmT_bf = np.ascontiguousarray(Bm.T).astype(ml_dtypes.bfloat16)
    in_maps = []
    for c in range(NCORES):
        in_maps.append(
            {
                "a_shard": A[MPC * c:MPC * (c + 1)],
                "bT": np.ascontiguousarray(np.roll(BmT_bf, -MPC * c, axis=1)),
                "b_diag": Bm[MPC * c:MPC * (c + 1)],
            }
        )
    global LAST_RESULTS
    kw = {}
    if TRACE:
        kw = {"trace": True, "tmpdir": TRACE_DIR}
    r = run_bass_kernel_spmd(nc, in_maps, list(range(NCORES)), **kw)
    LAST_RESULTS = r
    res = r.results
    # losses come back partition-major: value at [p, m] is row m*128+p
    losses = np.concatenate(
        [res[c]["losses"].reshape(128, NA).T.reshape(-1) for c in range(NCORES)]
    )
    return np.float32(np.mean(losses.astype(np.float64)))


# revision 27
# speedup vs baseline: 1.1200x; 1.1200x over previous
"""CircleLoss (nn_CircleLoss_55482387529741) Trainium2 Bass kernel.

Math (B=8192, D=128, margin m=0.25, gamma=256=16^2):
  a = l2norm(A) rows, b = l2norm(B) rows, s_ij = a_i . b_j
  logit_neg = g*relu(s-m)*(s+m)  ==>  exp(logit_neg) = exp(max(16*s, 4)^2 - 16)
  lse_pos_i = (w-12)(w-4) with w = min(16*s_ii, 12)
  loss_i = softplus(lse_pos_i + log(sum_{j!=i} exp(logit_neg_ij)))
  out = mean(loss)

Distribution: a-rows sharded 8 x 1024 across cores. Each core computes a
[8192 x 1024] "flipped" sim slab (partitions = b-rows, free = its a-rows).
B is rotated per core on host so the diagonal lands in local b-blocks 0..7.

Per 128-row b-block: PE matmul -> psum r; then one of two elementwise routes:
  route D (DVE): one fused custom op emits bf16 BITS of exp(z-16) via the
    Schraudolph trick: i16 = round(sq(max(r*invb, 4))*A16 + (B16-16*A16));
    bitcast i16 -> bf16 is e. One DVE pass, no ACT.
  route A (ACT): Relu(r*invb-4) -> Square(y+4) -> Exp(q-16), exact, 3 passes.
Routes are split ~47/17 to run DVE and ACT at full duty in parallel; the PE
ones-matmul accumulates row sums S over all 64 blocks (216 ns/512-col issue
rate, LDW hidden). b-norm prep: GPSIMD squares (prep window only - its SBUF
port contends with DVE), PE selector-matmul partition reduce, short DRAM
roundtrip. No GPSIMD work once the hot loop runs. Epilogue ln() uses the
inverse bit trick (bias-corrected); S reshape via PE row transposes and the
losses go out partition-major (host reorders) - no 4-byte scatter DMAs.
"""

import sys

for _p in ("/opt/trn_rl_repo",):
    if _p not in sys.path:
        sys.path.append(_p)

import numpy as np

import concourse.bass as bass
from concourse import bacc
import concourse.mybir as mybir
import concourse.tile as tile
from concourse.bass_utils import run_bass_kernel_spmd
from concourse.masks import make_identity

F32 = mybir.dt.float32
BF16 = mybir.dt.bfloat16
I16 = mybir.dt.int16
I32 = mybir.dt.int32
AF = mybir.ActivationFunctionType
OP = mybir.AluOpType

B = 8192
D = 128
NCORES = 8
MPC = B // NCORES  # 1024 a-rows per core
NB = B // 128  # 64 b-blocks
NA = MPC // 128  # 8 a-tiles

A16 = 128.0 / float(np.log(2.0))  # bf16 Schraudolph scale
BOFF = 127.0 * 128.0 - 16.0 * A16  # bits offset incl. the -16 in exp(z-16)
A32 = float(2**23) / float(np.log(2.0))
LNC = 127.0 * 2**23 / A32  # subtract after bits/A32
LN_BIAS = 0.0397  # E[ln(1+m) - m*ln2] over uniform mantissa

# blocks handled by the exact ACT route (rest use the DVE bit-trick route).
# diag blocks 0..7 stay on DVE so the antieye fix never crosses engines.
N_ACT = 19
ACT_BLOCKS = sorted({8 + int(round(i * 55 / max(N_ACT - 1, 1))) for i in range(N_ACT)})

_cache = {}

# test-harness hooks (default off; kernel() stays self-contained)
TRACE = False
TRACE_DIR = None
LAST_RESULTS = None


def _get_custom_op():
    """out_i16 = round(sq(maxx(in0*s0, s1))*imm2 + C3), C3 latched from in1."""
    from concourse import dve_ops
    from concourse.dve_spec import (
        Spec, Src0, C0, C1, C2, C3, maxx, sq, lower, _spill_c3_to_src1,
    )
    from concourse.dve_spec import _has_src1 as has_src1
    from concourse.dve_uop import DveOpSpec

    name = "SCHRAUDOLPH_CIRCLE"
    for o in dve_ops.OPS:
        if o.name == name:
            return o

    def _ref(in0, in1, s0, s1, imm2):
        z = np.square(
            np.maximum(in0.astype(np.float32) * np.float32(s0), np.float32(s1))
        )
        return (z * np.float32(imm2) + in1.astype(np.float32)).astype(np.float32)

    spec = Spec(
        body=_spill_c3_to_src1(sq(maxx(Src0 * C0, C1)) * C2 + C3), reference=_ref
    )
    opcode = dve_ops._CUSTOM_DVE_ROW_BASE + len(dve_ops.OPS)
    assert opcode < 0x20
    shas = {}
    for ver in ("v3", "v4"):
        try:
            shas[ver] = DveOpSpec(
                name=name,
                opcode=opcode,
                uops=lower(spec, ver=ver),
                rd1_en=has_src1(spec),
            ).sha(ver)
        except Exception:
            pass
    op = dve_ops.DveOp(name, spec, subdim=False, uops_sha=shas)
    dve_ops.OPS.append(op)
    dve_ops.CUSTOM_DVE_SPECS[name] = spec
    dve_ops._SUB_OPCODE_FOR_NAME[name] = opcode
    return op


def _build():
    if "nc" in _cache:
        return _cache["nc"]
    op = _get_custom_op()
    nc = bacc.Bacc("TRN2", target_bir_lowering=False)

    a_in = nc.declare_dram_parameter("a_shard", [MPC, D], F32, isOutput=False)
    bT_in = nc.declare_dram_parameter("bT", [D, B], BF16, isOutput=False)
    bd_in = nc.declare_dram_parameter("b_diag", [MPC, D], BF16, isOutput=False)
    out = nc.declare_dram_parameter("losses", [MPC], F32, isOutput=True)
    ssb_scr = nc.dram_tensor("ssb_scratch", [B], F32)
    out_pm = out.rearrange("(p m) -> p m", p=128)  # contiguous per partition

    with tile.TileContext(nc) as tc:
        with (
            tc.tile_pool(name="consts", bufs=1) as consts,
            tc.tile_pool(name="big", bufs=1) as big,
            tc.tile_pool(name="aprep", bufs=1) as aprep,
            tc.tile_pool(name="bsq", bufs=2) as bsqp,
            tc.tile_pool(name="epool", bufs=5) as epool,
            tc.tile_pool(name="ypool", bufs=2) as ypool,
            tc.tile_pool(name="stats", bufs=1) as stats,
            tc.tile_pool(name="psim", bufs=3, space="PSUM") as psim,
            tc.tile_pool(name="psacc", bufs=1, space="PSUM") as psacc,
        ):
            # ---- DMAs: all queued sequentially on sync - the queue-stuff
            # serialization (~0.7us each) staggers the transfers so the
            # early, critical ones (a_big, bT0-3) win the HBM bandwidth.
            a_big = aprep.tile([128, NA, D], F32, tag="a_stage")
            nc.sync.dma_start(
                out=a_big, in_=a_in.rearrange("(i p) d -> p i d", p=128)
            )
            bT = big.tile([128, B], BF16, tag="bT")
            for g in range(4):
                nc.sync.dma_start(
                    out=bT[:, g * 1024:(g + 1) * 1024],
                    in_=bT_in[:, g * 1024:(g + 1) * 1024],
                )
            b_diag = aprep.tile([128, NA, D], BF16, tag="b_diag")
            nc.sync.dma_start(
                out=b_diag, in_=bd_in.rearrange("(i p) d -> p i d", p=128)
            )
            for g in range(4, 8):
                nc.sync.dma_start(
                    out=bT[:, g * 1024:(g + 1) * 1024],
                    in_=bT_in[:, g * 1024:(g + 1) * 1024],
                )

            # ---- constants ----
            eye = consts.tile([128, 128], BF16, tag="eye")
            make_identity(nc, eye)
            antieye = consts.tile([128, 128], I16, tag="antieye")
            nc.vector.tensor_scalar(
                out=antieye, in0=eye, scalar1=-1.0, scalar2=1.0,
                op0=OP.mult, op1=OP.add,
            )
            ones = consts.tile([128, 1], BF16, tag="ones")
            nc.vector.memset(ones, 1.0)
            one1 = consts.tile([1, 1], F32, tag="one1")
            nc.vector.memset(one1, 1.0)
            bcol = consts.tile([128, 1], F32, tag="bcol")
            nc.vector.memset(bcol, BOFF)
            b_m4 = consts.tile([128, 1], F32, tag="b_m4")
            nc.vector.memset(b_m4, -4.0)
            b_p4 = consts.tile([128, 1], F32, tag="b_p4")
            nc.vector.memset(b_p4, 4.0)
            b_m16 = consts.tile([128, 1], F32, tag="b_m16")
            nc.vector.memset(b_m16, -16.0)
            wsel = consts.tile([128, 1024], BF16, tag="wsel")
            nc.vector.memset(wsel, 0.0)
            for g in range(4):
                nc.vector.memset(wsel[:, g * 128 + g:g * 128 + g + 1], 1.0)

            # ---- persistent tensors ----
            aT = big.tile([128, MPC], BF16, tag="aT")
            invb = stats.tile([128, NB], F32, tag="invb")
            rd = stats.tile([128, NA], F32, tag="rd")

            # ---- a norms: one big Square (ACT) + reduce/recip (DVE) + one
            # Sqrt (ACT); then per-tile bf16 scale (DVE) + transpose (PE).
            asq = aprep.tile([128, NA, D], F32, tag="a_sq")
            nc.scalar.activation(out=asq, in_=a_big, func=AF.Square)
            ssa = stats.tile([128, NA], F32, tag="ssa")
            nc.vector.tensor_reduce(
                out=ssa, in_=asq, axis=mybir.AxisListType.X, op=OP.add
            )
            ra = stats.tile([128, NA], F32, tag="ra")
            nc.vector.reciprocal(out=ra, in_=ssa)
            inva16 = stats.tile([128, NA], F32, tag="inva16")
            nc.scalar.activation(out=inva16, in_=ra, func=AF.Sqrt, scale=256.0)

            bsqs = []
            for g in range(8):
                bsq = bsqp.tile([128, 1024], BF16, tag="b_sq")
                bsqs.append(bsq)

            def bsq_dve(g):
                nc.vector.tensor_mul(
                    bsqs[g], bT[:, g * 1024:(g + 1) * 1024],
                    bT[:, g * 1024:(g + 1) * 1024],
                )

            a16 = aprep.tile([128, NA, D], BF16, tag="a16")

            def a16_mul(i):
                nc.vector.tensor_mul(
                    a16[:, i, :], a_big[:, i, :],
                    inva16[:, i:i + 1].to_broadcast((128, D)),
                )

            # DVE order: interleave the b squares with the a16 scales so the
            # invb and aT chains advance together.
            bsq_dve(0)
            bsq_dve(1)
            for i in range(3):
                a16_mul(i)
            bsq_dve(2)
            for i in range(3, 6):
                a16_mul(i)
            bsq_dve(3)
            a16_mul(6)
            a16_mul(7)

            pt = psim.tile([128, 1024], BF16, tag="sim")
            for i in range(NA):
                nc.tensor.transpose(
                    pt[:, i * 128:(i + 1) * 128], a16[:, i, :], eye
                )

            # ---- b-norm partition reduce: selector matmuls on PE ----
            ssb_ps = psim.tile([128, 1024], F32, tag="sim")

            def wk_mm(g, start, stop):
                # selector column g%4 -> chunk sums land on psum row g%4
                q = g % 4
                for h in range(2):
                    nc.tensor.matmul(
                        ssb_ps[:, h * 512:(h + 1) * 512],
                        wsel[:, q * 128:(q + 1) * 128],
                        bsqs[g][:, h * 512:(h + 1) * 512],
                        start=start, stop=stop, skip_group_check=True,
                    )

            for g in range(4):
                wk_mm(g, start=(g == 0), stop=(g == 3))

            # S accumulator region; its partitions 1-127 are borrowed for
            # the invb chunk-0 reshape transposes before the first ones-mm.
            Sbig = psacc.tile([128, MPC], F32, tag="S")
            S_ps = Sbig[0:1, :]

            # ACT: invb-chunk copy first (gates the hot loop), then aT.
            ssbA_sb = stats.tile([4, 1024], F32, tag="ssbA_sb")
            nc.scalar.copy(out=ssbA_sb, in_=ssb_ps[0:4, :])
            nc.scalar.copy(out=aT[:, 0:512], in_=pt[:, 0:512])
            nc.scalar.copy(out=aT[:, 512:1024], in_=pt[:, 512:1024])
            # invb for blocks 0-7 via 8 PE row transposes (no DRAM gather)
            for m in range(8):
                nc.tensor.transpose(
                    Sbig[:, m:m + 1], ssbA_sb[0:1, m * 128:(m + 1) * 128], one1
                )
            ssbA_pm0 = stats.tile([128, 8], F32, tag="ssbA_pm0")
            nc.vector.tensor_scalar(
                out=ssbA_pm0, in0=Sbig[:, 0:8], scalar1=1.0, scalar2=None,
                op0=OP.mult,
            )
            # rd = rowsum(a16 * b_diag): raw diagonal sim (both bf16); runs
            # in the DVE's dependency-bound idle gap before the first custom
            rdp = aprep.tile([128, NA, D], F32, tag="rdp")
            nc.vector.tensor_mul(rdp, a16, b_diag)
            nc.vector.tensor_reduce(
                out=rd, in_=rdp, axis=mybir.AxisListType.X, op=OP.add
            )
            rbA0 = stats.tile([128, 8], F32, tag="rbA0")
            nc.vector.reciprocal(out=rbA0, in_=ssbA_pm0)
            nc.scalar.activation(out=invb[:, 0:8], in_=rbA0, func=AF.Sqrt)
            # blocks 8-31 via the DRAM roundtrip (latency hidden)
            nc.sync.dma_start(
                out=ssb_scr[0:4096].rearrange("(q j) -> q j", q=4), in_=ssbA_sb
            )
            ssbA_pm1 = stats.tile([128, 24], F32, tag="ssbA_pm1")
            nc.sync.dma_start(
                out=ssbA_pm1,
                in_=ssb_scr[1024:4096].rearrange("(m p) -> p m", p=128),
            )
            rbA1 = stats.tile([128, 24], F32, tag="rbA1")
            nc.vector.reciprocal(out=rbA1, in_=ssbA_pm1)
            nc.scalar.activation(out=invb[:, 8:32], in_=rbA1, func=AF.Sqrt)

            # second-half b squares on ACT (its pre-hot-loop idle tail)
            for g in range(4, 8):
                nc.scalar.activation(
                    out=bsqs[g], in_=bT[:, g * 1024:(g + 1) * 1024],
                    func=AF.Square,
                )
            for g in range(4, 8):
                wk_mm(g, start=(g == 4), stop=(g == 7))
            ssbB_sb = stats.tile([4, 1024], F32, tag="ssbB_sb")
            nc.scalar.copy(out=ssbB_sb, in_=ssb_ps[0:4, :])
            nc.sync.dma_start(
                out=ssb_scr[4096:8192].rearrange("(q j) -> q j", q=4),
                in_=ssbB_sb,
            )
            ssbB_pm = stats.tile([128, 32], F32, tag="ssbB_pm")
            nc.sync.dma_start(
                out=ssbB_pm,
                in_=ssb_scr[4096:8192].rearrange("(m p) -> p m", p=128),
            )

            act_set = set(ACT_BLOCKS)
            movs = {}

            def elementwise(m, ps):
                if m in act_set:
                    y = ypool.tile([128, MPC], F32, tag="y")
                    nc.scalar.activation(
                        out=y, in_=ps, func=AF.Relu,
                        scale=invb[:, m:m + 1], bias=b_m4,
                    )
                    q = ypool.tile([128, MPC], F32, tag="q")
                    nc.scalar.activation(out=q, in_=y, func=AF.Square, bias=b_p4)
                    e = epool.tile([128, MPC], BF16, tag="ea")
                    nc.scalar.activation(out=e, in_=q, func=AF.Exp, bias=b_m16)
                    movs[m] = e
                else:
                    eb = epool.tile([128, MPC], I16, tag="eb")
                    nc.vector._custom_dve(
                        op, out=eb, in0=ps, in1=bcol,
                        s0=invb[:, m:m + 1], s1=4.0, imm2=A16,
                    )
                    if m < NA:
                        sl = slice(m * 128, (m + 1) * 128)
                        nc.vector.tensor_mul(eb[:, sl], eb[:, sl], antieye)
                    movs[m] = eb.bitcast(BF16)

            # ---- main loop over 64 b-blocks, ones-matmuls batched by 4 ----
            for grp in range(16):
                ms = range(grp * 4, grp * 4 + 4)
                if grp == 1:
                    # second half of b-norm prep; data is ready long before
                    # the engine streams reach this point, so no stall. The
                    # throwaway Exp preloads the exp table set ahead of the
                    # first ACT-route block.
                    rbB = stats.tile([128, 32], F32, tag="rbB")
                    nc.vector.reciprocal(out=rbB, in_=ssbB_pm)
                    nc.scalar.activation(out=invb[:, 32:64], in_=rbB, func=AF.Sqrt)
                    warm = stats.tile([128, 1], F32, tag="warm")
                    nc.scalar.activation(out=warm, in_=bcol, func=AF.Exp, scale=0.0)
                for m in ms:
                    ps = psim.tile([128, MPC], F32, tag="sim")
                    for h in range(2):
                        nc.tensor.matmul(
                            ps[:, h * 512:(h + 1) * 512],
                            bT[:, m * 128:(m + 1) * 128],
                            aT[:, h * 512:(h + 1) * 512],
                            start=True, stop=True,
                        )
                    elementwise(m, ps)
                for m in ms:
                    mov = movs.pop(m)
                    for h in range(2):
                        nc.tensor.matmul(
                            S_ps[:, h * 512:(h + 1) * 512],
                            ones,
                            mov[:, h * 512:(h + 1) * 512],
                            start=(m == 0), stop=(m == NB - 1),
                            skip_group_check=True,
                        )

            # ---- epilogue: per-row losses ----
            S_sb = stats.tile([1, MPC], F32, tag="S_sb")
            nc.scalar.copy(out=S_sb, in_=S_ps)
            # reshape [1, 1024] -> [128, 8] with 8 PE row transposes (no DMA)
            sT = psim.tile([128, 1024], F32, tag="sim")
            for m in range(NA):
                nc.tensor.transpose(
                    sT[:, m:m + 1], S_sb[0:1, m * 128:(m + 1) * 128], one1
                )
            Srs = stats.tile([128, NA], F32, tag="Srs")
            nc.scalar.copy(out=Srs, in_=sT[:, 0:NA])
            # lse = ln(S) via inverse bit trick (+mean-bias fix folded into +48)
            lse = stats.tile([128, NA], F32, tag="lse")
            nc.vector.tensor_scalar(
                out=lse, in0=Srs.bitcast(I32), scalar1=1.0 / A32, scalar2=-LNC,
                op0=OP.mult, op1=OP.add,
            )
            sdiag = stats.tile([128, NA], F32, tag="sdiag")
            nc.vector.tensor_mul(sdiag, rd, invb[:, 0:NA])
            w = stats.tile([128, NA], F32, tag="w")
            nc.vector.tensor_scalar(
                out=w, in0=sdiag, scalar1=12.0, scalar2=None, op0=OP.min
            )
            lpr = stats.tile([128, NA], F32, tag="lpr")
            nc.vector.scalar_tensor_tensor(
                out=lpr, in0=w, scalar=16.0, in1=w, op0=OP.subtract, op1=OP.mult
            )
            t = stats.tile([128, NA], F32, tag="t")
            nc.vector.scalar_tensor_tensor(
                out=t, in0=lpr, scalar=48.0 + LN_BIAS, in1=lse,
                op0=OP.add, op1=OP.add,
            )
            abst = stats.tile([128, NA], F32, tag="abst")
            nc.scalar.activation(out=abst, in_=t, func=AF.Abs)
            u = stats.tile([128, NA], F32, tag="u")
            nc.scalar.activation(out=u, in_=abst, func=AF.Exp, scale=-1.0)
            up1 = stats.tile([128, NA], F32, tag="up1")
            nc.vector.tensor_scalar(
                out=up1, in0=u, scalar1=1.0, scalar2=None, op0=OP.add
            )
            v = stats.tile([128, NA], F32, tag="v")
            nc.vector.tensor_scalar(
                out=v, in0=up1.bitcast(I32), scalar1=1.0 / A32, scalar2=-LNC,
                op0=OP.mult, op1=OP.add,
            )
            loss = stats.tile([128, NA], F32, tag="loss")
            nc.vector.scalar_tensor_tensor(
                out=loss, in0=t, scalar=0.0, in1=v, op0=OP.max, op1=OP.add
            )
            nc.sync.dma_start(out=out_pm, in_=loss)

    nc.finalize()
    _cache["nc"] = nc
    return nc


def kernel(embeddings_a: np.ndarray, embeddings_b: np.ndarray) -> np.ndarray:
    import ml_dtypes

    nc = _build()
    A = np.ascontiguousarray(embeddings_a, dtype=np.float32)
    Bm = np.ascontiguousarray(embeddings_b, dtype=np.float32)
    BmT_bf = np.ascontiguousarray(Bm.T).astype(ml_dtypes.bfloat16)
    Bm_bf = Bm.astype(ml_dtypes.bfloat16)
    in_maps = []
    for c in range(NCORES):
        in_maps.append(
            {
                "a_shard": A[MPC * c:MPC * (c + 1)],
                "bT": np.ascontiguousarray(np.roll(BmT_bf, -MPC * c, axis=1)),
                "b_diag": Bm_bf[MPC * c:MPC * (c + 1)],
            }
        )
    global LAST_RESULTS
    kw = {}
    if TRACE:
        kw = {"trace": True, "tmpdir": TRACE_DIR}
    r = run_bass_kernel_spmd(nc, in_maps, list(range(NCORES)), **kw)
    LAST_RESULTS = r
    res = r.results
    # losses come back partition-major: value at [p, m] is row m*128+p
    losses = np.concatenate(
        [res[c]["losses"].reshape(128, NA).T.reshape(-1) for c in range(NCORES)]
    )
    return np.float32(np.mean(losses.astype(np.float64)))


# revision 28
# speedup vs baseline: 1.1609x; 1.0365x over previous
"""CircleLoss (nn_CircleLoss_55482387529741) Trainium2 Bass kernel.

Math (B=8192, D=128, margin m=0.25, gamma=256=16^2):
  a = l2norm(A) rows, b = l2norm(B) rows, s_ij = a_i . b_j
  logit_neg = g*relu(s-m)*(s+m)  ==>  exp(logit_neg) = exp(max(16*s, 4)^2 - 16)
  lse_pos_i = (w-12)(w-4) with w = min(16*s_ii, 12)
  loss_i = softplus(lse_pos_i + log(sum_{j!=i} exp(logit_neg_ij)))
  out = mean(loss)

Distribution: a-rows sharded 8 x 1024 across cores. Each core computes a
[8192 x 1024] "flipped" sim slab (partitions = b-rows, free = its a-rows).
B is rotated per core on host so the diagonal lands in local b-blocks 0..7.

Per 128-row b-block: PE matmul -> psum r; then one of two elementwise routes:
  route D (DVE): one fused custom op emits bf16 BITS of exp(z-16) via the
    Schraudolph trick: i16 = round(sq(max(r*invb, 4))*A16 + (B16-16*A16));
    bitcast i16 -> bf16 is e. One DVE pass, no ACT.
  route A (ACT): Relu(r*invb-4) -> Square(y+4) -> Exp(q-16), exact, 3 passes.
Routes are split ~47/17 to run DVE and ACT at full duty in parallel; the PE
ones-matmul accumulates row sums S over all 64 blocks (216 ns/512-col issue
rate, LDW hidden). b-norm prep: GPSIMD squares (prep window only - its SBUF
port contends with DVE), PE selector-matmul partition reduce, short DRAM
roundtrip. No GPSIMD work once the hot loop runs. Epilogue ln() uses the
inverse bit trick (bias-corrected); S reshape via PE row transposes and the
losses go out partition-major (host reorders) - no 4-byte scatter DMAs.
"""

import sys

for _p in ("/opt/trn_rl_repo",):
    if _p not in sys.path:
        sys.path.append(_p)

import numpy as np

import concourse.bass as bass
from concourse import bacc
import concourse.mybir as mybir
import concourse.tile as tile
from concourse.bass_utils import run_bass_kernel_spmd
from concourse.masks import make_identity

F32 = mybir.dt.float32
BF16 = mybir.dt.bfloat16
I16 = mybir.dt.int16
I32 = mybir.dt.int32
AF = mybir.ActivationFunctionType
OP = mybir.AluOpType

B = 8192
D = 128
NCORES = 8
MPC = B // NCORES  # 1024 a-rows per core
NB = B // 128  # 64 b-blocks
NA = MPC // 128  # 8 a-tiles

A16 = 128.0 / float(np.log(2.0))  # bf16 Schraudolph scale
BOFF = 127.0 * 128.0 - 16.0 * A16  # bits offset incl. the -16 in exp(z-16)
A32 = float(2**23) / float(np.log(2.0))
LNC = 127.0 * 2**23 / A32  # subtract after bits/A32
LN_BIAS = 0.0397  # E[ln(1+m) - m*ln2] over uniform mantissa

# blocks handled by the exact ACT route (rest use the DVE bit-trick route).
# diag blocks 0..7 stay on DVE so the antieye fix never crosses engines.
N_ACT = 17
ACT_BLOCKS = sorted({8 + int(round(i * 55 / max(N_ACT - 1, 1))) for i in range(N_ACT)})

_cache = {}

# test-harness hooks (default off; kernel() stays self-contained)
TRACE = False
TRACE_DIR = None
LAST_RESULTS = None


def _get_custom_op():
    """out_i16 = round(sq(maxx(in0*s0, s1))*imm2 + C3), C3 latched from in1."""
    from concourse import dve_ops
    from concourse.dve_spec import (
        Spec, Src0, C0, C1, C2, C3, maxx, sq, lower, _spill_c3_to_src1,
    )
    from concourse.dve_spec import _has_src1 as has_src1
    from concourse.dve_uop import DveOpSpec

    name = "SCHRAUDOLPH_CIRCLE"
    for o in dve_ops.OPS:
        if o.name == name:
            return o

    def _ref(in0, in1, s0, s1, imm2):
        z = np.square(
            np.maximum(in0.astype(np.float32) * np.float32(s0), np.float32(s1))
        )
        return (z * np.float32(imm2) + in1.astype(np.float32)).astype(np.float32)

    spec = Spec(
        body=_spill_c3_to_src1(sq(maxx(Src0 * C0, C1)) * C2 + C3), reference=_ref
    )
    opcode = dve_ops._CUSTOM_DVE_ROW_BASE + len(dve_ops.OPS)
    assert opcode < 0x20
    shas = {}
    for ver in ("v3", "v4"):
        try:
            shas[ver] = DveOpSpec(
                name=name,
                opcode=opcode,
                uops=lower(spec, ver=ver),
                rd1_en=has_src1(spec),
            ).sha(ver)
        except Exception:
            pass
    op = dve_ops.DveOp(name, spec, subdim=False, uops_sha=shas)
    dve_ops.OPS.append(op)
    dve_ops.CUSTOM_DVE_SPECS[name] = spec
    dve_ops._SUB_OPCODE_FOR_NAME[name] = opcode
    return op


def _build():
    if "nc" in _cache:
        return _cache["nc"]
    op = _get_custom_op()
    nc = bacc.Bacc("TRN2", target_bir_lowering=False)

    a_in = nc.declare_dram_parameter("a_shard", [MPC, D], F32, isOutput=False)
    bT_in = nc.declare_dram_parameter("bT", [D, B], BF16, isOutput=False)
    bd_in = nc.declare_dram_parameter("b_diag", [MPC, D], BF16, isOutput=False)
    out = nc.declare_dram_parameter("losses", [MPC], F32, isOutput=True)
    ssb_scr = nc.dram_tensor("ssb_scratch", [B], F32)
    out_pm = out.rearrange("(p m) -> p m", p=128)  # contiguous per partition

    with tile.TileContext(nc) as tc:
        with (
            tc.tile_pool(name="consts", bufs=1) as consts,
            tc.tile_pool(name="big", bufs=1) as big,
            tc.tile_pool(name="aprep", bufs=1) as aprep,
            tc.tile_pool(name="bsq", bufs=2) as bsqp,
            tc.tile_pool(name="epool", bufs=5) as epool,
            tc.tile_pool(name="ypool", bufs=2) as ypool,
            tc.tile_pool(name="stats", bufs=1) as stats,
            tc.tile_pool(name="psim", bufs=3, space="PSUM") as psim,
            tc.tile_pool(name="psacc", bufs=1, space="PSUM") as psacc,
        ):
            # ---- DMAs: all queued sequentially on sync - the queue-stuff
            # serialization (~0.7us each) staggers the transfers so the
            # early, critical ones (a_big, bT0-3) win the HBM bandwidth.
            a_big = aprep.tile([128, NA, D], F32, tag="a_stage")
            nc.sync.dma_start(
                out=a_big, in_=a_in.rearrange("(i p) d -> p i d", p=128)
            )
            bT = big.tile([128, B], BF16, tag="bT")
            for g in range(4):
                nc.sync.dma_start(
                    out=bT[:, g * 1024:(g + 1) * 1024],
                    in_=bT_in[:, g * 1024:(g + 1) * 1024],
                )
            b_diag = aprep.tile([128, NA, D], BF16, tag="b_diag")
            nc.sync.dma_start(
                out=b_diag, in_=bd_in.rearrange("(i p) d -> p i d", p=128)
            )
            for g in range(4, 8):
                nc.sync.dma_start(
                    out=bT[:, g * 1024:(g + 1) * 1024],
                    in_=bT_in[:, g * 1024:(g + 1) * 1024],
                )

            # ---- constants ----
            eye = consts.tile([128, 128], BF16, tag="eye")
            make_identity(nc, eye)
            antieye = consts.tile([128, 128], I16, tag="antieye")
            nc.vector.tensor_scalar(
                out=antieye, in0=eye, scalar1=-1.0, scalar2=1.0,
                op0=OP.mult, op1=OP.add,
            )
            ones = consts.tile([128, 1], BF16, tag="ones")
            nc.vector.memset(ones, 1.0)
            one1 = consts.tile([1, 1], F32, tag="one1")
            nc.vector.memset(one1, 1.0)
            bcol = consts.tile([128, 1], F32, tag="bcol")
            nc.vector.memset(bcol, BOFF)
            b_m4 = consts.tile([128, 1], F32, tag="b_m4")
            nc.vector.memset(b_m4, -4.0)
            b_p4 = consts.tile([128, 1], F32, tag="b_p4")
            nc.vector.memset(b_p4, 4.0)
            b_m16 = consts.tile([128, 1], F32, tag="b_m16")
            nc.vector.memset(b_m16, -16.0)
            wsel = consts.tile([128, 1024], BF16, tag="wsel")
            nc.vector.memset(wsel, 0.0)
            for g in range(4):
                nc.vector.memset(wsel[:, g * 128 + g:g * 128 + g + 1], 1.0)

            # ---- persistent tensors ----
            aT = big.tile([128, MPC], BF16, tag="aT")
            invb = stats.tile([128, NB], F32, tag="invb")
            rd = stats.tile([128, NA], F32, tag="rd")

            # ---- a norms: one big Square (ACT) + reduce/recip (DVE) + one
            # Sqrt (ACT); then per-tile bf16 scale (DVE) + transpose (PE).
            asq = aprep.tile([128, NA, D], F32, tag="a_sq")
            nc.scalar.activation(out=asq, in_=a_big, func=AF.Square)
            ssa = stats.tile([128, NA], F32, tag="ssa")
            nc.vector.tensor_reduce(
                out=ssa, in_=asq, axis=mybir.AxisListType.X, op=OP.add
            )
            ra = stats.tile([128, NA], F32, tag="ra")
            nc.vector.reciprocal(out=ra, in_=ssa)
            inva16 = stats.tile([128, NA], F32, tag="inva16")
            nc.scalar.activation(out=inva16, in_=ra, func=AF.Sqrt, scale=256.0)

            bsqs = []
            for g in range(8):
                bsq = bsqp.tile([128, 1024], BF16, tag="b_sq")
                bsqs.append(bsq)

            def bsq_dve(g):
                nc.vector.tensor_mul(
                    bsqs[g], bT[:, g * 1024:(g + 1) * 1024],
                    bT[:, g * 1024:(g + 1) * 1024],
                )

            a16 = aprep.tile([128, NA, D], BF16, tag="a16")

            def a16_mul(i):
                nc.vector.tensor_mul(
                    a16[:, i, :], a_big[:, i, :],
                    inva16[:, i:i + 1].to_broadcast((128, D)),
                )

            # DVE order: interleave the b squares with the a16 scales so the
            # invb and aT chains advance together.
            bsq_dve(0)
            bsq_dve(1)
            for i in range(3):
                a16_mul(i)
            bsq_dve(2)
            for i in range(3, 6):
                a16_mul(i)
            bsq_dve(3)
            a16_mul(6)
            a16_mul(7)

            pt = psim.tile([128, 1024], BF16, tag="sim")
            for i in range(NA):
                nc.tensor.transpose(
                    pt[:, i * 128:(i + 1) * 128], a16[:, i, :], eye
                )

            # ---- b-norm partition reduce: selector matmuls on PE ----
            ssb_ps = psim.tile([128, 1024], F32, tag="sim")

            def wk_mm(g, start, stop):
                # selector column g%4 -> chunk sums land on psum row g%4
                q = g % 4
                for h in range(2):
                    nc.tensor.matmul(
                        ssb_ps[:, h * 512:(h + 1) * 512],
                        wsel[:, q * 128:(q + 1) * 128],
                        bsqs[g][:, h * 512:(h + 1) * 512],
                        start=start, stop=stop, skip_group_check=True,
                    )

            for g in range(4):
                wk_mm(g, start=(g == 0), stop=(g == 3))

            # S accumulator region; its partitions 1-127 are borrowed for
            # the invb chunk-0 reshape transposes before the first ones-mm.
            Sbig = psacc.tile([128, MPC], F32, tag="S")
            S_ps = Sbig[0:1, :]

            # ACT: invb-chunk copy first (gates the hot loop), then aT.
            ssbA_sb = stats.tile([4, 1024], F32, tag="ssbA_sb")
            nc.scalar.copy(out=ssbA_sb, in_=ssb_ps[0:4, :])
            nc.scalar.copy(out=aT[:, 0:512], in_=pt[:, 0:512])
            nc.scalar.copy(out=aT[:, 512:1024], in_=pt[:, 512:1024])
            # invb for blocks 0-7 via 8 PE row transposes (no DRAM gather)
            for m in range(8):
                nc.tensor.transpose(
                    Sbig[:, m:m + 1], ssbA_sb[0:1, m * 128:(m + 1) * 128], one1
                )
            ssbA_pm0 = stats.tile([128, 8], F32, tag="ssbA_pm0")
            nc.vector.tensor_scalar(
                out=ssbA_pm0, in0=Sbig[:, 0:8], scalar1=1.0, scalar2=None,
                op0=OP.mult,
            )
            # rd = rowsum(a16 * b_diag): raw diagonal sim (both bf16); runs
            # in the DVE's dependency-bound idle gap before the first custom
            rdp = aprep.tile([128, NA, D], F32, tag="rdp")
            nc.vector.tensor_mul(rdp, a16, b_diag)
            nc.vector.tensor_reduce(
                out=rd, in_=rdp, axis=mybir.AxisListType.X, op=OP.add
            )
            rbA0 = stats.tile([128, 8], F32, tag="rbA0")
            nc.vector.reciprocal(out=rbA0, in_=ssbA_pm0)
            nc.scalar.activation(out=invb[:, 0:8], in_=rbA0, func=AF.Sqrt)
            # blocks 8-31 via the DRAM roundtrip (latency hidden)
            nc.sync.dma_start(
                out=ssb_scr[0:4096].rearrange("(q j) -> q j", q=4), in_=ssbA_sb
            )
            ssbA_pm1 = stats.tile([128, 24], F32, tag="ssbA_pm1")
            nc.sync.dma_start(
                out=ssbA_pm1,
                in_=ssb_scr[1024:4096].rearrange("(m p) -> p m", p=128),
            )
            rbA1 = stats.tile([128, 24], F32, tag="rbA1")
            nc.vector.reciprocal(out=rbA1, in_=ssbA_pm1)
            nc.scalar.activation(out=invb[:, 8:32], in_=rbA1, func=AF.Sqrt)

            # second-half b squares on ACT (its pre-hot-loop idle tail)
            for g in range(4, 8):
                nc.scalar.activation(
                    out=bsqs[g], in_=bT[:, g * 1024:(g + 1) * 1024],
                    func=AF.Square,
                )
            for g in range(4, 8):
                wk_mm(g, start=(g == 4), stop=(g == 7))
            ssbB_sb = stats.tile([4, 1024], F32, tag="ssbB_sb")
            nc.scalar.copy(out=ssbB_sb, in_=ssb_ps[0:4, :])
            nc.sync.dma_start(
                out=ssb_scr[4096:8192].rearrange("(q j) -> q j", q=4),
                in_=ssbB_sb,
            )
            ssbB_pm = stats.tile([128, 32], F32, tag="ssbB_pm")
            nc.sync.dma_start(
                out=ssbB_pm,
                in_=ssb_scr[4096:8192].rearrange("(m p) -> p m", p=128),
            )

            act_set = set(ACT_BLOCKS)
            movs = {}

            def elementwise(m, ps):
                if m in act_set:
                    y = ypool.tile([128, MPC], F32, tag="y")
                    nc.scalar.activation(
                        out=y, in_=ps, func=AF.Relu,
                        scale=invb[:, m:m + 1], bias=b_m4,
                    )
                    q = ypool.tile([128, MPC], F32, tag="q")
                    nc.scalar.activation(out=q, in_=y, func=AF.Square, bias=b_p4)
                    e = epool.tile([128, MPC], BF16, tag="ea")
                    nc.scalar.activation(out=e, in_=q, func=AF.Exp, bias=b_m16)
                    movs[m] = e
                else:
                    eb = epool.tile([128, MPC], I16, tag="eb")
                    nc.vector._custom_dve(
                        op, out=eb, in0=ps, in1=bcol,
                        s0=invb[:, m:m + 1], s1=4.0, imm2=A16,
                    )
                    if m < NA:
                        sl = slice(m * 128, (m + 1) * 128)
                        nc.vector.tensor_mul(eb[:, sl], eb[:, sl], antieye)
                    movs[m] = eb.bitcast(BF16)

            # ---- main loop over 64 b-blocks, ones-matmuls batched by 2 ----
            for grp in range(32):
                ms = range(grp * 2, grp * 2 + 2)
                if grp == 2:
                    # second half of b-norm prep; data is ready long before
                    # the engine streams reach this point, so no stall. The
                    # throwaway Exp preloads the exp table set ahead of the
                    # first ACT-route block.
                    rbB = stats.tile([128, 32], F32, tag="rbB")
                    nc.vector.reciprocal(out=rbB, in_=ssbB_pm)
                    nc.scalar.activation(out=invb[:, 32:64], in_=rbB, func=AF.Sqrt)
                    warm = stats.tile([128, 1], F32, tag="warm")
                    nc.scalar.activation(out=warm, in_=bcol, func=AF.Exp, scale=0.0)
                for m in ms:
                    ps = psim.tile([128, MPC], F32, tag="sim")
                    for h in range(2):
                        nc.tensor.matmul(
                            ps[:, h * 512:(h + 1) * 512],
                            bT[:, m * 128:(m + 1) * 128],
                            aT[:, h * 512:(h + 1) * 512],
                            start=True, stop=True,
                        )
                    elementwise(m, ps)
                for m in ms:
                    mov = movs.pop(m)
                    for h in range(2):
                        nc.tensor.matmul(
                            S_ps[:, h * 512:(h + 1) * 512],
                            ones,
                            mov[:, h * 512:(h + 1) * 512],
                            start=(m == 0), stop=(m == NB - 1),
                            skip_group_check=True,
                        )

            # ---- epilogue: per-row losses ----
            S_sb = stats.tile([1, MPC], F32, tag="S_sb")
            nc.scalar.copy(out=S_sb, in_=S_ps)
            # reshape [1, 1024] -> [128, 8] with 8 PE row transposes (no DMA)
            sT = psim.tile([128, 1024], F32, tag="sim")
            for m in range(NA):
                nc.tensor.transpose(
                    sT[:, m:m + 1], S_sb[0:1, m * 128:(m + 1) * 128], one1
                )
            Srs = stats.tile([128, NA], F32, tag="Srs")
            nc.scalar.copy(out=Srs, in_=sT[:, 0:NA])
            # lse = ln(S) via inverse bit trick (+mean-bias fix folded into +48)
            lse = stats.tile([128, NA], F32, tag="lse")
            nc.vector.tensor_scalar(
                out=lse, in0=Srs.bitcast(I32), scalar1=1.0 / A32, scalar2=-LNC,
                op0=OP.mult, op1=OP.add,
            )
            sdiag = stats.tile([128, NA], F32, tag="sdiag")
            nc.vector.tensor_mul(sdiag, rd, invb[:, 0:NA])
            w = stats.tile([128, NA], F32, tag="w")
            nc.vector.tensor_scalar(
                out=w, in0=sdiag, scalar1=12.0, scalar2=None, op0=OP.min
            )
            lpr = stats.tile([128, NA], F32, tag="lpr")
            nc.vector.scalar_tensor_tensor(
                out=lpr, in0=w, scalar=16.0, in1=w, op0=OP.subtract, op1=OP.mult
            )
            t = stats.tile([128, NA], F32, tag="t")
            nc.vector.scalar_tensor_tensor(
                out=t, in0=lpr, scalar=48.0 + LN_BIAS, in1=lse,
                op0=OP.add, op1=OP.add,
            )
            abst = stats.tile([128, NA], F32, tag="abst")
            nc.scalar.activation(out=abst, in_=t, func=AF.Abs)
            u = stats.tile([128, NA], F32, tag="u")
            nc.scalar.activation(out=u, in_=abst, func=AF.Exp, scale=-1.0)
            up1 = stats.tile([128, NA], F32, tag="up1")
            nc.vector.tensor_scalar(
                out=up1, in0=u, scalar1=1.0, scalar2=None, op0=OP.add
            )
            v = stats.tile([128, NA], F32, tag="v")
            nc.vector.tensor_scalar(
                out=v, in0=up1.bitcast(I32), scalar1=1.0 / A32, scalar2=-LNC,
                op0=OP.mult, op1=OP.add,
            )
            loss = stats.tile([128, NA], F32, tag="loss")
            nc.vector.scalar_tensor_tensor(
                out=loss, in0=t, scalar=0.0, in1=v, op0=OP.max, op1=OP.add
            )
            nc.sync.dma_start(out=out_pm, in_=loss)

    nc.finalize()
    _cache["nc"] = nc
    return nc


def kernel(embeddings_a: np.ndarray, embeddings_b: np.ndarray) -> np.ndarray:
    import ml_dtypes

    nc = _build()
    A = np.ascontiguousarray(embeddings_a, dtype=np.float32)
    Bm = np.ascontiguousarray(embeddings_b, dtype=np.float32)
    BmT_bf = np.ascontiguousarray(Bm.T).astype(ml_dtypes.bfloat16)
    Bm_bf = Bm.astype(ml_dtypes.bfloat16)
    in_maps = []
    for c in range(NCORES):
        in_maps.append(
            {
                "a_shard": A[MPC * c:MPC * (c + 1)],
                "bT": np.ascontiguousarray(np.roll(BmT_bf, -MPC * c, axis=1)),
                "b_diag": Bm_bf[MPC * c:MPC * (c + 1)],
            }
        )
    global LAST_RESULTS
    kw = {}
    if TRACE:
        kw = {"trace": True, "tmpdir": TRACE_DIR}
    r = run_bass_kernel_spmd(nc, in_maps, list(range(NCORES)), **kw)
    LAST_RESULTS = r
    res = r.results
    # losses come back partition-major: value at [p, m] is row m*128+p
    losses = np.concatenate(
        [res[c]["losses"].reshape(128, NA).T.reshape(-1) for c in range(NCORES)]
    )
    return np.float32(np.mean(losses.astype(np.float64)))
